# revision 1
# baseline (speedup 1.0000x reference)
"""Bidirectional Mamba block on 8 trn2 NeuronCores (Bass/Tile).

Sharding: core c -> (batch = c//4, direction = (c%4)//2, d_inner half = c%2).
Each core computes a (768-channel) slice of one direction's Mamba for one
batch element, in transposed (channel-major) layout.  Cross-core comms:
  1. AllReduce (pairs) of the (80,1024) x-projection (dt_low|B|C).
  2. One ReduceScatter over the 4 cores of a batch (fwd half0/half1, bwd
     half0/half1) of the token-major partial fuse input; backward cores
     stage their rows pre-flipped (per-core flip-matrix input applied on
     PE within each 128-token block, plus partition_id-predicated DMAs
     that mirror the block destinations), so fwd+bwd partials align in
     original token coordinates and the RS sum directly yields the fused
     pre-LN input.  Each core then finishes layernorm/GELU/residual for
     its 256 tokens.

Engine layout: in_proj/out_proj/conv(diagonal-matmul)/scan-reduction on PE,
selective scan (DVE tensor_tensor_scan, 4 states chained per instruction),
dA exponentials on ACT, and all scan-region elementwise on DVE (measured on
HW: GPSIMD 2-input bf16 is ~3.3x slower than DVE and contends with the
scan for the shared SBUF port).
"""

import os
import sys

for _p in ("/opt/trn_rl_repo", "/opt/pypackages"):
    if _p not in sys.path:
        sys.path.insert(0, _p)

import numpy as np
import ml_dtypes
from contextlib import ExitStack

import concourse.bass as bass
import concourse.bacc as bacc
import concourse.tile as tile
import concourse.mybir as mybir

F32 = mybir.dt.float32
BF16 = mybir.dt.bfloat16
I32 = mybir.dt.int32
NPBF = ml_dtypes.bfloat16

DM = 768          # d_model
DIH = 768         # d_inner half (per core)
L = 1024
NST = 16          # d_state
DTR = 48          # dt_rank
KT = 6            # d_model / 128
DT = 6            # DIH / 128
MT = 12           # in_proj output tiles per core (6 xi + 6 z)
CH = 2            # 512-column chunks of L
LN_EPS = 1e-5
GRP = 4           # states chained per scan instruction
NG = NST // GRP   # scan groups per d-tile
TOK = 256         # output tokens per core after the 4-way ReduceScatter

AF = mybir.ActivationFunctionType
OP = mybir.AluOpType

PAIRS = [[0, 1], [2, 3], [4, 5], [6, 7]]          # same (batch, dir)
RSG = [[0, 1, 2, 3], [4, 5, 6, 7]]                # same batch (fwd+bwd)

# which scan groups run their bu / h*C multiply on GPSIMD instead of DVE.
# Measured on HW: GPSIMD 2-input bf16 is ~3.3x slower than DVE and contends
# for the shared SBUF port with DVE's scan -- keep everything on DVE.
BU_POOL_DEF = (False, False, False, False)
HC_POOL_DEF = (False, False, False, False)


def _declare(nc):
    t = {}

    def inp(name, shape, dt):
        t[name] = nc.dram_tensor(name, list(shape), dt, kind="ExternalInput")

    inp("xT", (DM, L), BF16)
    inp("xres", (TOK, DM), F32)
    inp("w_in", (MT, 128, KT * 128), BF16)
    inp("w_incs", (1, MT * 128), BF16)
    inp("w_cmb", (KT, 128, DM), BF16)
    inp("w_convd", (DT * 4, 128, 128), BF16)
    inp("w_Dd", (DT, 128, 128), BF16)
    inp("w_xp", (128, KT * 80), BF16)
    inp("w_dt", (DTR, DIH), BF16)
    inp("c_A", (128, DT * NST), F32)
    inp("c_convb", (128, DT), F32)
    inp("c_D", (128, DT), F32)
    inp("c_dtb", (128, DT), F32)
    inp("c_png", (128, KT), F32)
    inp("c_pnb", (128, KT), F32)
    inp("c_fb", (128, DM), F32)
    inp("c_flg", (128, DM), F32)
    inp("c_flb", (128, DM), F32)
    inp("c_id_bf", (128, 128), BF16)
    inp("c_flip", (128, 128), BF16)
    inp("c_ones_col_bf", (128, 1), BF16)
    inp("c_ones_row", (1, 128), F32)

    t["out"] = nc.dram_tensor("out", [TOK, DM], F32, kind="ExternalOutput")

    # internal DRAM staging
    t["st_dbc_a"] = nc.dram_tensor("st_dbc_a", [80, L], BF16)
    t["st_dbc_b"] = nc.dram_tensor("st_dbc_b", [80, L], BF16)
    t["st_pre"] = nc.dram_tensor("st_pre", [L, DM], BF16)
    t["st_rs"] = nc.dram_tensor("st_rs", [TOK, DM], BF16)
    return t


def _emit(ctx, tc, T, mock_collectives=False, loop_n=1, ln_trivial=True,
          fl_trivial=True, fb_trivial=True,
          bu_pool=BU_POOL_DEF, hc_pool=HC_POOL_DEF):
    BU_POOL, HC_POOL = bu_pool, hc_pool
    nc = tc.nc

    cpool = ctx.enter_context(tc.tile_pool(name="consts", bufs=1))
    wcp = ctx.enter_context(tc.tile_pool(name="p6w", bufs=1))

    def cload(name, shape, dt, eng=nc.sync):
        tl = cpool.tile(list(shape), dt, tag=name, name=name)
        eng.dma_start(out=tl[:], in_=T[name][:])
        return tl

    # tiny LN consts on the ACT queue (sub-us); x itself goes first on SP
    # inside the phase-1 block; cold consts are loaded mid-kernel, behind
    # the compute that precedes their first use.
    c_ones_col_bf = cload("c_ones_col_bf", (128, 1), BF16, eng=nc.scalar)
    c_ones_row = cload("c_ones_row", (1, 128), F32, eng=nc.scalar)
    c_png = cload("c_png", (128, KT), F32, eng=nc.scalar)
    c_pnb = cload("c_pnb", (128, KT), F32, eng=nc.scalar)
    c_eps = cpool.tile([128, 1], F32, tag="c_eps", name="c_eps")
    nc.vector.memset(c_eps[:], LN_EPS)

    if loop_n > 1:
        loop_cm = tc.For_i(0, loop_n, 1)
        loop_cm.__enter__()

    actA = ctx.enter_context(tc.tile_pool(name="actA", bufs=1))
    xc = [actA.tile([128, L], BF16, tag=f"xc{m}", name=f"xc{m}") for m in range(DT)]
    sz = [actA.tile([128, L], BF16, tag=f"sz{m}", name=f"sz{m}") for m in range(DT)]
    yg = [actA.tile([128, L], BF16, tag=f"yg{m}", name=f"yg{m}") for m in range(DT)]

    # ================= phases 1-3 (xn lifetime) ==========================
    with tc.tile_pool(name="actXN", bufs=1) as actXN:
        xtall = actXN.tile([128, KT * L], BF16, tag="xtall", name="xtall")
        nc.sync.dma_start(
            out=xtall[:],
            in_=bass.AP(T["xT"], 0, [[L, 128], [128 * L, KT], [1, L]]))
        xn = [actXN.tile([128, L], BF16, tag=f"xn{k}", name=f"xn{k}")
              for k in range(KT)]

        # ---- phase 1: layernorm of x (channel-major) --------------------
        with tc.tile_pool(name="p1tmp", bufs=2) as tmp1, \
             tc.tile_pool(name="p1mbrb", bufs=1) as mbrb, \
             tc.tile_pool(name="p1st", bufs=1) as stats, \
             tc.tile_pool(name="p1ps", bufs=1, space="PSUM") as ps_s, \
             tc.tile_pool(name="p1psb", bufs=2, space="PSUM") as ps_b:
            stat2 = ps_s.tile([33, L], F32, tag="stat2")
            xt = [xtall[:, k * L:(k + 1) * L] for k in range(KT)]

            psum_s = stat2[0:1, :]
            psum_q = stat2[32:33, :]
            for k in range(KT):
                sq = tmp1.tile([128, L], BF16, tag="sq")
                nc.scalar.square(sq[:], xt[k])
                for ch in range(CH):
                    sl = slice(ch * 512, (ch + 1) * 512)
                    nc.tensor.matmul(psum_s[:, sl], c_ones_col_bf[:],
                                     xt[k][:, sl],
                                     start=(k == 0), stop=(k == KT - 1))
                    nc.tensor.matmul(psum_q[:, sl], c_ones_col_bf[:], sq[:, sl],
                                     start=(k == 0), stop=(k == KT - 1))

            mean = stats.tile([1, L], F32, tag="mean")
            var = stats.tile([1, L], F32, tag="var")
            rstd = stats.tile([1, L], F32, tag="rstd")
            nc.scalar.mul(mean[:], psum_s[:], 1.0 / DM)
            m2 = stats.tile([1, L], F32, tag="m2")
            nc.scalar.square(m2[:], mean[:])
            nc.vector.scalar_tensor_tensor(var[:], psum_q[:], 1.0 / DM, m2[:],
                                           op0=OP.mult, op1=OP.subtract)
            std = stats.tile([1, L], F32, tag="std")
            nc.scalar.activation(std[:], var[:], AF.Sqrt, bias=c_eps[:1, :])
            nc.vector.reciprocal_approx_fast(rstd[:], std[:])

            # mr = mean*rstd row, folded into in_proj as a rank-1 K=1 matmul
            # (ln_trivial: xn = xt*rb; the -mean*rstd*colsum(W) correction is
            # appended to each in_proj accumulation chain)
            mr = actXN.tile([1, L], BF16, tag="mr", name="mr")
            nc.vector.tensor_mul(mr[:], mean[:], rstd[:])

            # broadcast rstd (and mean when g/b nontrivial) over partitions
            rb = mbrb.tile([128, L], BF16, tag="rb")
            if not ln_trivial:
                mb = mbrb.tile([128, L], BF16, tag="mb")
            for ch in range(CH):
                sl = slice(ch * 512, (ch + 1) * 512)
                pb2 = ps_b.tile([128, 512], F32, tag="pb")
                nc.tensor.matmul(pb2[:], c_ones_row[:], rstd[:, sl])
                nc.scalar.copy(rb[:, sl], pb2[:])
                if not ln_trivial:
                    pb = ps_b.tile([128, 512], F32, tag="pb")
                    nc.tensor.matmul(pb[:], c_ones_row[:], mean[:, sl])
                    nc.scalar.copy(mb[:, sl], pb[:])

            for k in range(KT):
                if ln_trivial:
                    nc.vector.tensor_mul(xn[k][:], xt[k], rb[:])
                else:
                    t1 = tmp1.tile([128, L], BF16, tag="t1")
                    nc.vector.tensor_sub(t1[:], xt[k], mb[:])
                    t2 = tmp1.tile([128, L], BF16, tag="t2")
                    nc.vector.tensor_mul(t2[:], t1[:], rb[:])
                    nc.vector.tensor_scalar(xn[k][:], t2[:],
                                            c_png[:, k:k + 1], c_pnb[:, k:k + 1],
                                            op0=OP.mult, op1=OP.add)

        # cold const loads, behind phase-1 compute in queue order
        c_convb = cload("c_convb", (128, DT), F32, eng=nc.scalar)
        cvd = cpool.tile([128, DT * 4 * 128], BF16, tag="cvd", name="cvd")
        nc.scalar.dma_start(
            out=cvd[:],
            in_=bass.AP(T["w_convd"], 0,
                        [[128, 128], [128 * 128, DT * 4], [1, 128]]))
        c_xp = cload("w_xp", (128, KT * 80), BF16, eng=nc.scalar)
        c_dtw = cload("w_dt", (DTR, DIH), BF16, eng=nc.scalar)
        c_incs = cload("w_incs", (1, MT * 128), BF16, eng=nc.scalar)

        # ---- phase 2/3: in_proj; xi half first so the dbc AllReduce can
        # ---- launch early, z half afterwards (overlaps the collective) --
        with tc.tile_pool(name="p2w", bufs=1) as wmp, \
             tc.tile_pool(name="p2pad", bufs=2) as padp, \
             tc.tile_pool(name="p3sb", bufs=1) as p3, \
             tc.tile_pool(name="p2ps", bufs=2, space="PSUM") as psA, \
             tc.tile_pool(name="p2cv", bufs=1, space="PSUM") as psC, \
             tc.tile_pool(name="p3ps", bufs=1, space="PSUM") as psD:

            # xi-half weights in one DMA; z-half loaded during the AllReduce
            wi = wmp.tile([128, DT * KT * 128], BF16, tag="wi", name="wi")
            wi_z = wmp.tile([128, DT * KT * 128], BF16, tag="wi_z", name="wi_z")
            nc.sync.dma_start(
                out=wi[:],
                in_=bass.AP(T["w_in"], 0,
                            [[KT * 128, 128], [128 * KT * 128, DT], [1, KT * 128]]))

            def in_proj_tile(mt):
                is_xi = mt < DT
                wsrc = wi if is_xi else wi_z
                wofs = (mt if is_xi else mt - DT) * KT * 128
                dest = None
                if is_xi:
                    dest = padp.tile([128, L + 3], BF16, tag="xipad",
                                     name="xipad")
                    nc.vector.memset(dest[:, 0:3], 0.0)
                for ch in range(CH):
                    pa = psA.tile([128, 512], F32, tag="pa", name="pa")
                    sl = slice(ch * 512, (ch + 1) * 512)
                    for k in range(KT):
                        nc.tensor.matmul(pa[:],
                                         wsrc[:, wofs + k * 128:wofs + (k + 1) * 128],
                                         xn[k][:, sl],
                                         start=(k == 0),
                                         stop=(k == KT - 1 and not ln_trivial))
                    if ln_trivial:
                        # rank-1 mean correction: pa -= colsum(W) x (mean*rstd)
                        nc.tensor.matmul(pa[:], c_incs[:, mt * 128:(mt + 1) * 128],
                                         mr[:, sl], start=False, stop=True)
                    if is_xi:
                        nc.scalar.copy(dest[:, 3 + ch * 512: 3 + (ch + 1) * 512],
                                       pa[:])
                    else:
                        nc.scalar.activation(
                            sz[mt - DT][:, ch * 512:(ch + 1) * 512],
                            pa[:], AF.Silu)
                if is_xi:
                    # causal depthwise conv as 4 diagonal matmuls on PE
                    pc = psC.tile([128, L], F32, tag="pc", name="pc")
                    for ch in range(CH):
                        for tap in range(4):
                            nc.tensor.matmul(
                                pc[:, ch * 512:(ch + 1) * 512],
                                cvd[:, (mt * 4 + tap) * 128:(mt * 4 + tap + 1) * 128],
                                dest[:, tap + ch * 512: tap + ch * 512 + 512],
                                start=(tap == 0), stop=(tap == 3))
                    nc.scalar.activation(xc[mt][:], pc[:], AF.Silu,
                                         bias=c_convb[:, mt:mt + 1])

            for mt in range(DT):          # xi half + conv
                in_proj_tile(mt)

            # x-projection (dt_low|B|C) + pair AllReduce
            pd = psD.tile([80, L], F32, tag="pd")
            for ch in range(CH):
                sl = slice(ch * 512, (ch + 1) * 512)
                for k in range(KT):
                    nc.tensor.matmul(pd[:, sl], c_xp[:, k * 80:(k + 1) * 80],
                                     xc[k][:, sl],
                                     start=(k == 0), stop=(k == KT - 1))
            dbc_half = p3.tile([80, L], BF16, tag="dbc_half")
            nc.scalar.copy(dbc_half[:], pd[:])
            nc.sync.dma_start(out=T["st_dbc_a"][:], in_=dbc_half[:])
            if mock_collectives:
                nc.sync.dma_start(out=T["st_dbc_b"][:], in_=T["st_dbc_a"][:])
            else:
                nc.gpsimd.collective_compute(
                    "AllReduce", OP.add, replica_groups=PAIRS,
                    ins=[T["st_dbc_a"][:].opt()], outs=[T["st_dbc_b"][:].opt()])

            # z-half weight load + matmuls overlap the AllReduce, as do the
            # remaining cold const loads (scan/out-proj phase inputs)
            nc.sync.dma_start(
                out=wi_z[:],
                in_=bass.AP(T["w_in"], DT * 128 * KT * 128,
                            [[KT * 128, 128], [128 * KT * 128, DT], [1, KT * 128]]))
            c_dtb = cload("c_dtb", (128, DT), F32)
            c_A = cload("c_A", (128, DT * NST), F32)
            c_id_bf = cload("c_id_bf", (128, 128), BF16)
            c_flip = cload("c_flip", (128, 128), BF16)
            cdd = cpool.tile([128, DT * 128], BF16, tag="cdd", name="cdd")
            nc.sync.dma_start(
                out=cdd[:],
                in_=bass.AP(T["w_Dd"], 0,
                            [[128, 128], [128 * 128, DT], [1, 128]]))
            wcs = wcp.tile([128, KT * DM], BF16, tag="wcs", name="wcs")
            nc.sync.dma_start(
                out=wcs[:],
                in_=bass.AP(T["w_cmb"], 0, [[DM, 128], [128 * DM, KT], [1, DM]]))
            for mt in range(DT, MT):      # z half (overlaps the AllReduce)
                in_proj_tile(mt)

    # ================= phases 4-5 (B/C + dt/g lifetime) ==================
    with tc.tile_pool(name="actBC", bufs=1) as actBC:
        bball = actBC.tile([128, NST * L], BF16, tag="bball")
        cball = actBC.tile([128, NST * L], BF16, tag="cball")
        dtlow = actBC.tile([DTR, L], BF16, tag="dtlow")
        dtt = [actBC.tile([128, L], BF16, tag=f"dt{m}", name=f"dt{m}")
               for m in range(DT)]
        gg = [actBC.tile([128, L], BF16, tag=f"g{m}", name=f"g{m}")
              for m in range(DT)]

        # broadcast-load B and C rows (partition-step-0 DMA), plus dt_low;
        # all on the SP queue (ACT queue DMAs would stall softplus)
        nc.sync.dma_start(out=dtlow[:], in_=T["st_dbc_b"][0:DTR, :])
        bb3 = bball[:].rearrange("p (n t) -> p n t", n=NST)
        cb3 = cball[:].rearrange("p (n t) -> p n t", n=NST)
        for g in range(NG):
            src_b = bass.AP(T["st_dbc_b"], (DTR + g * GRP) * L,
                            [[0, 128], [L, GRP], [1, L]])
            src_c = bass.AP(T["st_dbc_b"], (DTR + NST + g * GRP) * L,
                            [[0, 128], [L, GRP], [1, L]])
            nc.sync.dma_start(out=bb3[:, g * GRP:(g + 1) * GRP, :], in_=src_b)
            nc.sync.dma_start(out=cb3[:, g * GRP:(g + 1) * GRP, :], in_=src_c)

        # ---- phase 4: dt projection + softplus + g ----------------------
        # softplus(x) = ln(1 + exp(x)); all Exp passes first, then all Ln
        # passes in-place (one act-table switch instead of one per pair)
        with tc.tile_pool(name="p4ps", bufs=2, space="PSUM") as psT:
            for m in range(DT):
                for ch in range(CH):
                    sl = slice(ch * 512, (ch + 1) * 512)
                    pt = psT.tile([128, 512], F32, tag="pt")
                    nc.tensor.matmul(pt[:], c_dtw[:, m * 128:(m + 1) * 128],
                                     dtlow[:, sl])
                    nc.scalar.activation(dtt[m][:, sl], pt[:], AF.Exp,
                                         bias=c_dtb[:, m:m + 1])
            for m in range(DT):
                nc.scalar.activation(dtt[m][:], dtt[m][:], AF.Ln, bias=1.0)
                nc.vector.tensor_mul(gg[m][:], dtt[m][:], xc[m][:])

        # ---- phase 5: selective scan ------------------------------------
        # GRP states per scan instruction; the decay (dA) column at each
        # state boundary is zeroed so the chained recurrence resets.
        GL = GRP * L
        with tc.tile_pool(name="p5da", bufs=2) as dap, \
             tc.tile_pool(name="p5bu", bufs=2) as bup, \
             tc.tile_pool(name="p5h", bufs=2) as hp, \
             tc.tile_pool(name="p5ps", bufs=2, space="PSUM") as psY:
            for m in range(DT):
                py = psY.tile([128, L], F32, tag="py")
                for g in range(NG):
                    dag = dap.tile([128, GL], BF16, tag="dag", name="dag")
                    for j in range(GRP):
                        n = g * GRP + j
                        nc.scalar.activation(
                            dag[:, j * L:(j + 1) * L], dtt[m][:], AF.Exp,
                            scale=c_A[:, m * NST + n:m * NST + n + 1])
                    # zero the first decay column of every state block
                    dz = dag[:].rearrange("p (n t) -> p n t", n=GRP)[:, :, 0:1]
                    nc.vector.memset(dz, 0.0)

                    bu = bup.tile([128, GL], BF16, tag="bu", name="bu")
                    grep = bass.AP(gg[m][:].tensor, 0,
                                   [[L, 128], [0, GRP], [1, L]])
                    bsl = bball[:, g * GL:(g + 1) * GL]
                    beng = nc.gpsimd if BU_POOL[g] else nc.vector
                    beng.tensor_tensor(
                        bu[:].rearrange("p (n t) -> p n t", n=GRP),
                        grep, bsl.rearrange("p (n t) -> p n t", n=GRP),
                        op=OP.mult)

                    hc = dag            # reuse the consumed dag tile for h*C
                    h = hp.tile([128, GL], BF16, tag="h", name="h")
                    nc.vector.tensor_tensor_scan(h[:], dag[:], bu[:], 0.0,
                                                 op0=OP.mult, op1=OP.add)
                    heng = nc.gpsimd if HC_POOL[g] else nc.vector
                    heng.tensor_mul(hc[:], h[:],
                                    cball[:, g * GL:(g + 1) * GL])
                    for q in range(GL // 512):
                        sl_in = slice(q * 512, (q + 1) * 512)
                        sl_out = slice((q * 512) % L, (q * 512) % L + 512)
                        nc.tensor.matmul(py[:, sl_out], c_id_bf[:], hc[:, sl_in],
                                         start=(g == 0 and q < CH),
                                         stop=False,
                                         skip_group_check=True)
                # y = py + xc*D via diag(D) matmul into the same PSUM group;
                # then y_gated = y * silu(z)
                for ch in range(CH):
                    sl = slice(ch * 512, (ch + 1) * 512)
                    nc.tensor.matmul(py[:, sl],
                                     cdd[:, m * 128:(m + 1) * 128],
                                     xc[m][:, sl], start=False, stop=True,
                                     skip_group_check=True)
                nc.vector.tensor_mul(yg[m][:], py[:], sz[m][:])

    # ================= phase 6: out-proj, token-major via stationary yg ==
    # out[tok, dm] = sum_k yg[k][:, tok_chunk]^T @ w_cmb[k]; the token
    # chunk is the PE stationary operand so the PSUM result is already
    # token-major.  The backward direction's token flip is resolved with a
    # per-core permutation input (c_flip = I fwd / J bwd) applied on PE,
    # plus a predicated pair of block-destination DMAs (partition_id cond).
    pid = nc.sync.partition_id()
    is_bwd = (pid & 2) != 0
    is_fwd = (pid & 2) == 0
    with tc.tile_pool(name="p6ps", bufs=2, space="PSUM") as psF, \
         tc.tile_pool(name="p6psf", bufs=2, space="PSUM") as psFf, \
         tc.tile_pool(name="p6sb", bufs=2) as p6:
        for tcn in range(8):
            pf = psF.tile([128, DM], F32, tag="pf", name="pf")
            tsl = slice(tcn * 128, (tcn + 1) * 128)
            for k in range(KT):
                nc.tensor.matmul(pf[:, 0:512], yg[k][:, tsl],
                                 wcs[:, k * DM:k * DM + 512],
                                 start=(k == 0), stop=(k == KT - 1),
                                 skip_group_check=True)
                nc.tensor.matmul(pf[:, 512:DM], yg[k][:, tsl],
                                 wcs[:, k * DM + 512:(k + 1) * DM],
                                 start=(k == 0), stop=(k == KT - 1),
                                 skip_group_check=True)
            fu = p6.tile([128, DM], BF16, tag="fu", name="fu")
            nc.scalar.copy(fu[:], pf[:])
            pxf = psFf.tile([128, DM], F32, tag="pxf", name="pxf")
            nc.tensor.matmul(pxf[:, 0:512], c_flip[:], fu[:, 0:512])
            nc.tensor.matmul(pxf[:, 512:DM], c_flip[:], fu[:, 512:DM])
            fu2 = p6.tile([128, DM], BF16, tag="fu2", name="fu2")
            nc.scalar.copy(fu2[:], pxf[:])
            nc.sync.dma_start(out=T["st_pre"][tcn * 128:(tcn + 1) * 128, :],
                              in_=fu2[:], cond=is_fwd)
            nc.sync.dma_start(out=T["st_pre"][(7 - tcn) * 128:(8 - tcn) * 128, :],
                              in_=fu2[:], cond=is_bwd)

    # ================= phase 7: RS + fused LN/GELU + residual ============
    if mock_collectives:
        nc.sync.dma_start(out=T["st_rs"][:], in_=T["st_pre"][0:TOK, :])
    else:
        nc.gpsimd.collective_compute(
            "ReduceScatter", OP.add, replica_groups=RSG,
            ins=[T["st_pre"][:].opt()], outs=[T["st_rs"][:].opt()])

    with tc.tile_pool(name="p7sb", bufs=1) as p7, \
         tc.tile_pool(name="p7c", bufs=1) as p7c:
        if not fb_trivial:
            fb = p7c.tile([128, DM], F32, tag="fb")
            nc.sync.dma_start(out=fb[:], in_=T["c_fb"][:])
        if not fl_trivial:
            flg = p7c.tile([128, DM], F32, tag="flg")
            nc.sync.dma_start(out=flg[:], in_=T["c_flg"][:])
            flb = p7c.tile([128, DM], F32, tag="flb")
            nc.sync.dma_start(out=flb[:], in_=T["c_flb"][:])

        xhs = []
        for tb in range(TOK // 128):
            nat = p7.tile([128, DM], BF16, tag=f"nat{tb}", name=f"nat{tb}")
            nc.sync.dma_start(out=nat[:],
                              in_=T["st_rs"][tb * 128:(tb + 1) * 128, :])
            if fb_trivial:
                pre = nat
            else:
                pre = p7.tile([128, DM], F32, tag=f"pre{tb}", name=f"pre{tb}")
                nc.vector.tensor_add(pre[:], nat[:], fb[:])

            red = p7.tile([128, 1], F32, tag="red")
            nc.vector.reduce_sum(red[:], pre[:], axis=mybir.AxisListType.X)
            mean = p7.tile([128, 1], F32, tag="mean")
            nc.vector.tensor_scalar_mul(mean[:], red[:], 1.0 / DM)
            sq = p7.tile([128, DM], F32, tag="sq")
            nc.scalar.square(sq[:], pre[:])
            red2 = p7.tile([128, 1], F32, tag="red2")
            nc.vector.reduce_sum(red2[:], sq[:], axis=mybir.AxisListType.X)
            m2 = p7.tile([128, 1], F32, tag="m2")
            nc.scalar.square(m2[:], mean[:])
            var = p7.tile([128, 1], F32, tag="var")
            nc.vector.scalar_tensor_tensor(var[:], red2[:], 1.0 / DM, m2[:],
                                           op0=OP.mult, op1=OP.subtract)
            std = p7.tile([128, 1], F32, tag="std")
            nc.scalar.activation(std[:], var[:], AF.Sqrt, bias=c_eps[:])
            rstd = p7.tile([128, 1], F32, tag="rstd")
            nc.vector.reciprocal_approx_fast(rstd[:], std[:])

            xh = p7.tile([128, DM], F32, tag=f"xh{tb}", name=f"xh{tb}")
            nc.vector.tensor_scalar(xh[:], pre[:], mean[:], rstd[:],
                                    op0=OP.subtract, op1=OP.mult)
            if not fl_trivial:
                xg = p7.tile([128, DM], F32, tag="xg")
                nc.vector.tensor_mul(xg[:], xh[:], flg[:])
                nc.vector.tensor_add(xh[:], xg[:], flb[:])
            xhs.append(xh)

        # all Sqrt activations above, all Gelu below: 2 table loads total
        for tb in range(TOK // 128):
            ge = p7.tile([128, DM], F32, tag=f"ge{tb}", name=f"ge{tb}")
            nc.scalar.activation(ge[:], xhs[tb][:], AF.Gelu_apprx_tanh)
            res = p7.tile([128, DM], F32, tag="res")
            nc.sync.dma_start(out=res[:],
                              in_=T["xres"][tb * 128:(tb + 1) * 128, :])
            fin = p7.tile([128, DM], F32, tag="fin")
            nc.vector.tensor_add(fin[:], ge[:], res[:])
            nc.sync.dma_start(out=T["out"][tb * 128:(tb + 1) * 128, :],
                              in_=fin[:])

    if loop_n > 1:
        loop_cm.__exit__(None, None, None)


_CACHE = {}


def build_program(mock_collectives=False, loop_n=1, num_devices=None,
                  ln_trivial=True, fl_trivial=True, fb_trivial=True,
                  bu_pool=BU_POOL_DEF, hc_pool=HC_POOL_DEF):
    key = ("prog", mock_collectives, loop_n, num_devices, ln_trivial,
           fl_trivial, fb_trivial, tuple(bu_pool), tuple(hc_pool))
    if key in _CACHE:
        return _CACHE[key]
    nd = num_devices if num_devices is not None else (1 if mock_collectives else 8)
    nc = bacc.Bacc("TRN2", target_bir_lowering=False, num_devices=nd)
    T = _declare(nc)
    with tile.TileContext(nc) as tc:
        with ExitStack() as ctx:
            _emit(ctx, tc, T, mock_collectives=mock_collectives, loop_n=loop_n,
                  ln_trivial=ln_trivial, fl_trivial=fl_trivial,
                  fb_trivial=fb_trivial,
                  bu_pool=tuple(bu_pool), hc_pool=tuple(hc_pool))
    nc.compile()
    _CACHE[key] = nc
    return nc


def _tile6(v):
    return np.ascontiguousarray(v.reshape(6, 128).T.astype(np.float32))


def make_in_maps(inputs):
    x = np.asarray(inputs["x"], np.float32)
    in_maps = []
    shared = {
        "c_png": _tile6(np.asarray(inputs["pn_g"], np.float32)),
        "c_pnb": _tile6(np.asarray(inputs["pn_b"], np.float32)),
        "c_fb": np.ascontiguousarray(
            np.broadcast_to(np.asarray(inputs["fuse_b"], np.float32), (128, DM))),
        "c_flg": np.ascontiguousarray(
            np.broadcast_to(np.asarray(inputs["fl_g"], np.float32), (128, DM))),
        "c_flb": np.ascontiguousarray(
            np.broadcast_to(np.asarray(inputs["fl_b"], np.float32), (128, DM))),
        "c_id_bf": np.eye(128, dtype=NPBF),
        "c_ones_col_bf": np.ones((128, 1), NPBF),
        "c_ones_row": np.ones((1, 128), np.float32),
    }
    fuse_W = np.asarray(inputs["fuse_W"], np.float32)

    flip_fwd = np.eye(128, dtype=NPBF)
    flip_bwd = np.ascontiguousarray(flip_fwd[::-1])

    for c in range(8):
        b = c // 4
        bwd = ((c % 4) // 2) == 1
        h = c % 2
        q = c % 4
        pfx = "b_" if bwd else "f_"
        rows = slice(DIH * h, DIH * h + DIH)

        inW = np.asarray(inputs[pfx + "inW"], np.float32)
        w_in_slice = np.concatenate(
            [inW[:, DIH * h:DIH * h + DIH],
             inW[:, 1536 + DIH * h:1536 + DIH * h + DIH]], axis=1)  # (768,1536)
        w_in = (w_in_slice.reshape(6, 128, 12, 128)
                .transpose(2, 1, 0, 3).reshape(12, 128, 768).astype(NPBF))
        w_incs = np.ascontiguousarray(
            (-w_in_slice.sum(axis=0)).reshape(1, MT * 128).astype(NPBF))

        outW = np.asarray(inputs[pfx + "outW"], np.float32)
        fuse_half = fuse_W[DM:] if bwd else fuse_W[:DM]
        w_c = outW[rows, :] @ fuse_half  # (768, 768), d_inner-half x d_model
        w_cmb = np.ascontiguousarray(w_c.reshape(6, 128, DM).astype(NPBF))

        xpW = np.asarray(inputs[pfx + "xpW"], np.float32)[rows]
        w_xp = (xpW.reshape(6, 128, 80).transpose(1, 0, 2)
                .reshape(128, 480).astype(NPBF))
        w_dt = np.ascontiguousarray(
            np.asarray(inputs[pfx + "dtW"], np.float32)[:, rows].astype(NPBF))

        A = -np.exp(np.asarray(inputs[pfx + "Alog"], np.float32)[rows])
        c_Aa = (A.reshape(6, 128, 16).transpose(1, 0, 2)
                .reshape(128, 96).astype(np.float32))
        convW = np.asarray(inputs[pfx + "convW"], np.float32)[rows]
        # (6 m-tiles, 4 taps) -> diagonal [128,128] matrices
        w_convd = np.zeros((24, 128, 128), np.float32)
        cw = convW.reshape(6, 128, 4)
        for m in range(6):
            for tap in range(4):
                np.fill_diagonal(w_convd[m * 4 + tap], cw[m, :, tap])
        w_convd = w_convd.astype(NPBF)
        Dv = np.asarray(inputs[pfx + "D"], np.float32)[rows].reshape(6, 128)
        w_Dd = np.zeros((6, 128, 128), np.float32)
        for m in range(6):
            np.fill_diagonal(w_Dd[m], Dv[m])
        w_Dd = w_Dd.astype(NPBF)

        xin = x[b]
        if bwd:
            xin = xin[::-1]
        xT = np.ascontiguousarray(xin.T.astype(NPBF))
        xres = np.ascontiguousarray(x[b, TOK * q:TOK * q + TOK])

        m = dict(shared)
        m.update({
            "xT": xT,
            "xres": xres,
            "w_in": np.ascontiguousarray(w_in),
            "w_incs": w_incs,
            "w_Dd": np.ascontiguousarray(w_Dd),
            "w_cmb": w_cmb,
            "w_convd": np.ascontiguousarray(w_convd),
            "w_xp": np.ascontiguousarray(w_xp),
            "w_dt": w_dt,
            "c_A": np.ascontiguousarray(c_Aa),
            "c_convb": _tile6(np.asarray(inputs[pfx + "convb"], np.float32)[rows]),
            "c_D": _tile6(np.asarray(inputs[pfx + "D"], np.float32)[rows]),
            "c_dtb": _tile6(np.asarray(inputs[pfx + "dtb"], np.float32)[rows]),
            "c_flip": flip_bwd if bwd else flip_fwd,
        })
        in_maps.append(m)
    return in_maps


def _flags(inputs):
    ln_trivial = (np.all(np.asarray(inputs["pn_g"]) == 1.0)
                  and np.all(np.asarray(inputs["pn_b"]) == 0.0))
    fl_trivial = (np.all(np.asarray(inputs["fl_g"]) == 1.0)
                  and np.all(np.asarray(inputs["fl_b"]) == 0.0))
    fb_trivial = np.all(np.asarray(inputs["fuse_b"]) == 0.0)
    return ln_trivial, fl_trivial, fb_trivial


def assemble(results):
    out = np.empty((2, L, DM), np.float32)
    for c in range(8):
        b = c // 4
        q = c % 4
        out[b, TOK * q:TOK * q + TOK] = results[c]["out"]
    return out


def kernel(**inputs):
    from concourse.bass_utils import run_bass_kernel_spmd
    ln_trivial, fl_trivial, fb_trivial = _flags(inputs)
    nc = build_program(ln_trivial=ln_trivial, fl_trivial=fl_trivial,
                       fb_trivial=fb_trivial)
    in_maps = make_in_maps(inputs)
    res = run_bass_kernel_spmd(nc, in_maps, list(range(8)))
    return assemble(res.results)

